# revision 35
# baseline (speedup 1.0000x reference)
"""Trainium2 Bass kernel for block-neighbor "contamination" stencil.

Problem: x [B=8, C=32, H=512, W=512] f32, kernel_size k=8.
The image is a 64x64 grid of 8x8 blocks. For each block, out = 0.8*block +
0.2 * mean(8 neighboring blocks) elementwise over the 8x8 tile, with
zero-padding of the block grid and per-position valid-neighbor counts
(interior 8, edges 5, corners 3).

Equivalent pixel form: a sparse 3x3 stencil with taps at +-8 pixels:
    out[r,w] = 0.8*x[r,w] + beta(r,w) * nsum[r,w]
    nsum[r,w] = sum over (dr,dw) in {-8,0,8}^2, (dr,dw) != (0,0), of
                x[r+dr, w+dw]  (zero pad at image borders)
    beta(r,w) = 0.2 / count(r,w),  count = Nr*Nw - 1,
    Nr/Nw = 2 at the first/last block row/col, else 3.

Strategy (pure data parallel, 1 batch item per NeuronCore, 8 cores).
The problem is HBM-bound (measured: DMA 92% busy at f32 I/O), so I/O is
done in bf16: the host casts x to bf16 (16 MiB/core in), the kernel
writes bf16 (16 MiB/core out), and the host upcasts to f32. End-to-end
rel err ~2.4e-3 (quantization), well within tolerance. This halves HBM
traffic: 32 MiB/core @ ~358 GB/s -> ~94 us floor vs 187 us at f32.

  * Layout: SBUF partition p = (channel-pair, block-row bi); free dim =
    (u = row-within-block 0..7, w 0..511). One partition = one block-row =
    8 consecutive image rows = 8KB contiguous DRAM at bf16.
  * Vertical block-neighbor taps (rows +-8) land at partition +-1 ->
    TensorEngine banded 128x128 matmuls (block-diagonal per channel),
    PSUM f32 accumulation. Horizontal taps are +-8 shifts along w via
    shifted moving-operand access patterns. beta(row) folded into the
    stationary weights; the center (0,0) tap with weight 0.8 is ALSO
    folded into the unshifted stationary (wc), so PSUM holds the full
    result for interior block-cols and the combine is a pure copy.
  * The copy PSUM->SBUF(bf16) runs on ScalarE (sits closest to PSUM;
    VectorE from PSUM is 1 elem/cycle and would become the bottleneck).
  * w-edge block-cols (first/last 8 columns) need the neighbor term
    rescaled by gamma(row) = (3Nr-1)/(2Nr-1): one tiny diagonal matmul
    per side (2-D moving AP over all 4 u-slices, FD=32) accumulates
    -0.8*(gamma-1)/gamma * x into the edge columns of PSUM, so
    out_edge = gamma * PSUM there -- one per-partition-scaled DVE copy
    (tensor_scalar_mul) per strip.
  * Matmuls are batched per stationary (4x wc, 8x wf, 2x we per
    half-tile) so LoadStationary switches hide under the PE's reorder
    window; FD-512 matmuls stream back-to-back at ~213ns.
  * HAM warmup: ~20 dummy matmuls on a GpSimd-memset scratch tile run
    in the startup dead window (7-12us) so the PE clock gate (default
    K=4/8 half rate until ~4us of sustained activity) is fully open
    when chunk 0 lands; chunk 0 loads as two half tiles to land early.
  * Store triggers are deferred past the next chunk's first PSUM copy
    (ScalarE is strict FIFO; a 0.6us DMA trigger ahead of the copy
    that releases the PSUM buffer stalls the PE), and the last chunk
    stores per half to shorten the drain.
  * Input DMAs ride the qSyncDynamicHW ring, output DMAs + weight
    loads the qScalarDynamicHW ring, so loads and stores stream
    concurrently.

Measured on HW: 205us (f32 baseline) -> ~113.5-116.7us run-to-run,
DMA ~93us active (the HBM/DMA-engine floor for 33.6MB at bf16), PE
~92us busy, rel err 2.53e-3 vs the f32 reference (gate 2e-2).
"""

import numpy as np

import concourse.mybir as mybir
import concourse.tile as tile
from concourse import bacc
from concourse.bass_utils import run_bass_kernel_spmd

import ml_dtypes

_BF16 = ml_dtypes.bfloat16

# Problem constants (hardcoded per harness contract).
B, C, H, W = 8, 32, 512, 512
K = 8  # block size
P = 128  # SBUF partitions
NBR = H // K  # 64 block-rows per channel
CPP = P // NBR  # channels per partition-tile (2)
N_CORES = 8
N_CHUNKS = C // CPP  # 16 tiles per core
HALF = K // 2  # u-slices per PSUM tile (4 banks)

BETA_INT = 0.2 / 8.0  # interior block-rows, interior block-cols
BETA_EDGE = 0.2 / 5.0  # edge block-rows, interior block-cols
GAMMA_INT = 8.0 / 5.0  # count ratio (3*Nr-1)/(2*Nr-1) at Nr=3
GAMMA_EDGE = 5.0 / 3.0  # at Nr=2

_EDGE_PARTS = (0, NBR - 1, NBR, P - 1)  # block-row 0/63 of each channel


def _make_weights():
    """Banded stationary matrices (vertical taps at partition +-1),
    block-diagonal per channel, beta folded in per output partition.
    wc additionally carries the 0.8 center tap. w1/w2 are the w-edge
    column corrections (beta_edge - beta) * (V - I) and * V: applied to
    the first/last 8 columns they retarget the neighbor weights from
    the interior count to the edge count, so PSUM holds the exact
    result everywhere and no post-scaling is needed."""
    beta = np.full(P, BETA_INT, np.float32)
    beta[list(_EDGE_PARTS)] = BETA_EDGE
    beta_e = np.full(P, 0.2 / 5.0, np.float32)  # Nw=2: count=2*Nr-1
    beta_e[list(_EDGE_PARTS)] = 0.2 / 3.0
    dbeta = beta_e - beta

    wf = np.zeros((P, P), np.float32)  # 3-band incl diag: applied to s
    wc = np.zeros((P, P), np.float32)  # 2-band + 0.8 center: unshifted
    w1 = np.zeros((P, P), np.float32)  # dbeta * (V - I): edge cols on x
    w2 = np.zeros((P, P), np.float32)  # dbeta * V: edge cols, x shifted
    for m in range(P):
        for d in (-1, 0, 1):
            k = m + d
            if 0 <= k < P and k // NBR == m // NBR:
                wf[k, m] = beta[m]
                w2[k, m] = dbeta[m]
                if d != 0:
                    wc[k, m] = beta[m]
                    w1[k, m] = dbeta[m]
        wc[m, m] = 0.8
    # pack all four stationaries into one [P, 4P] tensor: a single DMA
    # trigger (~0.7us on the scalar ring) instead of four, so the last
    # weight's completion receipt lands ~2us earlier at startup
    wall = np.concatenate([wf, wc, w1, w2], axis=1)
    return {"wall": wall.astype(_BF16)}


def _build_program():
    f32 = mybir.dt.float32
    bf16 = mybir.dt.bfloat16

    nc = bacc.Bacc("TRN2", target_bir_lowering=False, debug=False,
                   num_devices=N_CORES)

    x_dram = nc.dram_tensor("x", [C, H, W], bf16, kind="ExternalInput")
    y_dram = nc.dram_tensor("y", [C, H, W], bf16, kind="ExternalOutput")
    wall_dram = nc.dram_tensor("wall", [P, 4 * P], bf16, kind="ExternalInput")

    # partition axis = (channel, block-row); free = (u, w)
    x_v = x_dram[:].rearrange("c (bi u) w -> (c bi) u w", u=K)
    y_v = y_dram[:].rearrange("c (bi u) w -> (c bi) u w", u=K)

    with tile.TileContext(nc) as tc:
        with (
            tc.tile_pool(name="wpool", bufs=1) as wpool,
            tc.tile_pool(name="sbuf", bufs=4) as sbuf,
            tc.tile_pool(name="psum", bufs=2, space="PSUM") as psum,
        ):
            # weight loads ride the scalar (store) ring, which is idle
            # at startup, so the sync ring serves chunk loads only
            wall_t = wpool.tile([P, 4 * P], bf16, tag="wall")
            nc.scalar.dma_start(wall_t[:], wall_dram[:])
            wf_t = wall_t[:, 0 * P : 1 * P]
            wc_t = wall_t[:, 1 * P : 2 * P]
            w1_t = wall_t[:, 2 * P : 3 * P]
            w2_t = wall_t[:, 3 * P : 4 * P]

            # HAM warmup: the PE clock gate holds K=4/8 (half rate)
            # until ~4us of sustained activity. Zero the scratch on the
            # GpSimd queue (ready at ~4us, vs ~10us for ScalarE which
            # sits behind DGE config ops) so warmup matmuls run in the
            # ~7-11us window and real matmuls start at full 2.4 GHz.
            scratch = wpool.tile([P, W], bf16, tag="scratch")
            nc.gpsimd.memset(scratch[:], 0.0)
            # 48 LDW+MM pairs at the cold ~107ns pair rate ~= 4.9us of
            # CONTINUOUS activity from ~7.4us: past the ~4.6us HAM
            # unthrottle threshold, and long enough that the PE rolls
            # straight from warmup into chunk-0 work (~11.5us) with no
            # idle gap -- an idle gap decays the HAM credit and with the
            # lighter 2-pass PE schedule the gate then stays shut for
            # 10+us of half-rate matmuls
            uw = psum.tile([P, HALF, W], f32, tag="u")
            for _ in range(48):
                nc.tensor.matmul(uw[:, 0, 0:P], scratch[:, 0:P],
                                 scratch[:, 0:P], start=True, stop=True)

            pending_store = None
            for i in range(N_CHUNKS):
                p0 = i * P
                # steady-state loads stay on the qSyncDynamicHW ring:
                # mixing dependent stores into the same FIFO ring
                # head-of-line-blocks later loads. But the scalar HWDGE
                # ring is idle until the first store (~20us), so the
                # startup burst borrows it: chunk-0 h1 and chunk 1
                # stream there concurrently with chunk-0 h0 / chunk 2
                # on the sync ring, halving time-to-first-compute and
                # building 2 chunks of runahead before stores begin.
                if i == 0 or i == N_CHUNKS - 1:
                    # chunk 0: h1 borrows the idle scalar ring so both
                    # halves land concurrently (only wall+x01 ride the
                    # scalar ring early -- queuing more behind them
                    # delays x01's completion receipt by ~4us).
                    # chunk 15: halves on sync so h0's receipt lands
                    # ~2.5us earlier, shortening the compute tail.
                    xparts = []
                    for hh in range(2):
                        eng = nc.scalar if (i == 0 and hh == 1) else nc.sync
                        t = sbuf.tile([P, HALF, W], bf16, tag=f"x0{hh}")
                        eng.dma_start(
                            t[:],
                            x_v[p0 : p0 + P, hh * HALF : (hh + 1) * HALF],
                        )
                        xparts.append(t[:])
                else:
                    xin = sbuf.tile([P, K, W], bf16, tag="xin")
                    nc.sync.dma_start(xin[:], x_v[p0 : p0 + P])
                    xparts = [xin[:, 0:HALF], xin[:, HALF:K]]

                out_t = sbuf.tile([P, K, W], bf16, tag="out")
                for h in range(2):
                    u0 = h * HALF
                    xp = xparts[h]  # [P, HALF, W] view for this half
                    # separable stencil: the dj=+-1 taps applied to x
                    # equal the dj=0 taps applied to s = x<<8 + x>>8
                    # (zero-filled at the w edges). DVE builds s in bf16
                    # while the PE runs the wc/edge passes, so the PE
                    # does 2 full-width passes per u-slice instead of 3:
                    # per-chunk PE time drops 5.8us -> ~4.9us, below the
                    # 5.4us DMA period, so the PE no longer paces the
                    # pipeline. s depends only on the chunk load, and
                    # nothing else runs on DVE, so the s-builds stream
                    # ahead without coupling to the PSUM drain.
                    # early chunks arrive load-paced ~5.5-6us apart vs
                    # 4.9us of PE work, and the resulting boundary idle
                    # regularly trips a HAM close at ~18us (3.4us of
                    # half-rate matmuls). Two dependency-free dummies at
                    # the chunk top fill the idle: at this point the
                    # PSUM slot is already free (previous copy done) so
                    # they run while the chunk's data is still in
                    # flight; the wc pass's start=True discards them.
                    u = psum.tile([P, HALF, W], f32, tag="u")
                    if 1 <= i <= 4 and h == 0:
                        for _ in range(2):
                            nc.tensor.matmul(
                                u[:, 0, :], scratch[:, 0:P],
                                scratch[:], start=True, stop=True,
                            )
                    s = sbuf.tile([P, HALF, W], bf16, tag=f"s{h}")
                    nc.vector.tensor_add(
                        s[:, :, K : W - K],
                        xp[:, :, 0 : W - 2 * K],
                        xp[:, :, 2 * K : W],
                    )
                    nc.vector.tensor_copy(s[:, :, 0:K], xp[:, :, K : 2 * K])
                    nc.vector.tensor_copy(
                        s[:, :, W - K : W], xp[:, :, W - 2 * K : W - K]
                    )
                    for uu in range(HALF):
                        # center 0.8 tap + vertical taps, full width
                        nc.tensor.matmul(
                            u[:, uu, :], wc_t, xp[:, uu, :],
                            start=True, stop=False,
                        )
                    # w-edge column corrections: retarget the neighbor
                    # weights to the edge counts so PSUM ends up exact.
                    # Each matmul covers all 4 u-slices via a 2-D moving
                    # access pattern (FD=32); grouped per stationary so
                    # only 2 extra LoadStationary switches per half.
                    # These need only xp, giving DVE slack to finish s.
                    nc.tensor.matmul(
                        u[:, :, 0:K], w1_t, xp[:, :, 0:K],
                        start=False, stop=False,
                    )
                    nc.tensor.matmul(
                        u[:, :, W - K : W], w1_t,
                        xp[:, :, W - K : W],
                        start=False, stop=False,
                    )
                    nc.tensor.matmul(
                        u[:, :, 0:K], w2_t, xp[:, :, K : 2 * K],
                        start=False, stop=False,
                    )
                    nc.tensor.matmul(
                        u[:, :, W - K : W], w2_t,
                        xp[:, :, W - 2 * K : W - K],
                        start=False, stop=False,
                    )
                    for uu in range(HALF):
                        # all six dj=+-1 neighbor taps via s
                        nc.tensor.matmul(
                            u[:, uu, :], wf_t, s[:, uu, :],
                            start=False, stop=(uu == HALF - 1),
                        )
                    if i == N_CHUNKS - 1 and h == 1:
                        # pipeline drain for the very last half:
                        # quarter-sized copy->store pairs on the ScalarE
                        # FIFO so the first store's ~2us completion
                        # receipt overlaps the second quarter's copy
                        for q in range(0, HALF, 2):
                            nc.scalar.copy(
                                out_t[:, u0 + q : u0 + q + 2, :],
                                u[:, q : q + 2, :],
                            )
                            nc.scalar.dma_start(
                                y_v[p0 : p0 + P, u0 + q : u0 + q + 2],
                                out_t[:, u0 + q : u0 + q + 2],
                            )
                        continue
                    # result is complete in PSUM; pure full-width copy +
                    # f32->bf16 cast on ScalarE (closest to PSUM, and
                    # keeps DVE off the critical path)
                    nc.scalar.copy(
                        out_t[:, u0 : u0 + HALF, :],
                        u[:, :, :],
                    )
                    if i < 4:
                        # first chunks store per half right after the
                        # copy: the store stream starts ~6us earlier,
                        # shrinking the backlog that drains at
                        # single-ring rate after the loads finish (the
                        # early PE has slack to absorb the 0.65us
                        # trigger on the ScalarE FIFO)
                        nc.scalar.dma_start(
                            y_v[p0 : p0 + P, u0 : u0 + HALF],
                            out_t[:, u0 : u0 + HALF],
                        )
                    elif h == 0 and pending_store is not None:
                        # defer the previous chunk's store trigger to
                        # AFTER this half's copy: the copy (which gates
                        # PSUM buffer release -> PE) jumps the strict
                        # ScalarE FIFO ahead of the 0.6us DMA trigger
                        nc.scalar.dma_start(*pending_store)
                        pending_store = None
                    if i == N_CHUNKS - 1:
                        # drain the pipeline tail per half so the final
                        # store (and its ~2us completion receipt)
                        # starts as early as possible
                        nc.scalar.dma_start(
                            y_v[p0 : p0 + P, u0 : u0 + HALF],
                            out_t[:, u0 : u0 + HALF],
                        )
                # stores ride the scalar HWDGE ring so loads and stores
                # stream concurrently (the sync ring serializes behind
                # slot-paced load triggers; the gpsimd SW-DGE ring tops
                # out at ~130 GB/s -- both measured slower)
                if 4 <= i < N_CHUNKS - 1:
                    pending_store = (y_v[p0 : p0 + P], out_t[:])
    nc.compile()
    return nc


_CACHE = {}


def _get_program():
    if "nc" not in _CACHE:
        _CACHE["nc"] = _build_program()
        _CACHE["w"] = _make_weights()
    return _CACHE["nc"], _CACHE["w"]


def run(x, trace=False, **spmd_kwargs):
    """x: [B, C, H, W] f32 -> (results object, output [B, C, H, W] f32)."""
    nc, weights = _get_program()
    x = np.ascontiguousarray(x, dtype=np.float32).astype(_BF16)
    in_maps = [{"x": x[i], **weights} for i in range(N_CORES)]
    res = run_bass_kernel_spmd(nc, in_maps, list(range(N_CORES)),
                               trace=trace, **spmd_kwargs)
    out = np.stack([res.results[i]["y"] for i in range(N_CORES)], axis=0)
    return res, out.astype(np.float32)


def kernel(x, kernel_size=8, **_ignored):
    assert int(kernel_size) == K, f"kernel hardcoded for k={K}"
    x = np.asarray(x)
    assert x.shape == (B, C, H, W), x.shape
    _, out = run(x)
    return out


if __name__ == "__main__":
    rng = np.random.default_rng(0)
    x = rng.standard_normal((B, C, H, W), dtype=np.float32)
    out = kernel(x, 8)
    print("out", out.shape, out.dtype, float(np.abs(out).mean()))



# revision 36
# speedup vs baseline: 1.0408x; 1.0408x over previous
"""Trainium2 Bass kernel for block-neighbor "contamination" stencil.

Problem: x [B=8, C=32, H=512, W=512] f32, kernel_size k=8.
The image is a 64x64 grid of 8x8 blocks. For each block, out = 0.8*block +
0.2 * mean(8 neighboring blocks) elementwise over the 8x8 tile, with
zero-padding of the block grid and per-position valid-neighbor counts
(interior 8, edges 5, corners 3).

Equivalent pixel form: a sparse 3x3 stencil with taps at +-8 pixels:
    out[r,w] = 0.8*x[r,w] + beta(r,w) * nsum[r,w]
    nsum[r,w] = sum over (dr,dw) in {-8,0,8}^2, (dr,dw) != (0,0), of
                x[r+dr, w+dw]  (zero pad at image borders)
    beta(r,w) = 0.2 / count(r,w),  count = Nr*Nw - 1,
    Nr/Nw = 2 at the first/last block row/col, else 3.

Strategy (pure data parallel, 1 batch item per NeuronCore, 8 cores).
The problem is HBM-bound (measured: DMA 92% busy at f32 I/O), so I/O is
done in bf16: the host casts x to bf16 (16 MiB/core in), the kernel
writes bf16 (16 MiB/core out), and the host upcasts to f32. End-to-end
rel err ~2.4e-3 (quantization), well within tolerance. This halves HBM
traffic: 32 MiB/core @ ~358 GB/s -> ~94 us floor vs 187 us at f32.

  * Layout: SBUF partition p = (channel-pair, block-row bi); free dim =
    (u = row-within-block 0..7, w 0..511). One partition = one block-row =
    8 consecutive image rows = 8KB contiguous DRAM at bf16.
  * Vertical block-neighbor taps (rows +-8) land at partition +-1 ->
    TensorEngine banded 128x128 matmuls (block-diagonal per channel),
    PSUM f32 accumulation. Horizontal taps are +-8 shifts along w via
    shifted moving-operand access patterns. beta(row) folded into the
    stationary weights; the center (0,0) tap with weight 0.8 is ALSO
    folded into the unshifted stationary (wc), so PSUM holds the full
    result for interior block-cols and the combine is a pure copy.
  * The copy PSUM->SBUF(bf16) runs on ScalarE (sits closest to PSUM;
    VectorE from PSUM is 1 elem/cycle and would become the bottleneck).
  * w-edge block-cols (first/last 8 columns) need the neighbor term
    rescaled by gamma(row) = (3Nr-1)/(2Nr-1): one tiny diagonal matmul
    per side (2-D moving AP over all 4 u-slices, FD=32) accumulates
    -0.8*(gamma-1)/gamma * x into the edge columns of PSUM, so
    out_edge = gamma * PSUM there -- one per-partition-scaled DVE copy
    (tensor_scalar_mul) per strip.
  * Matmuls are batched per stationary (4x wc, 8x wf, 2x we per
    half-tile) so LoadStationary switches hide under the PE's reorder
    window; FD-512 matmuls stream back-to-back at ~213ns.
  * HAM warmup: ~20 dummy matmuls on a GpSimd-memset scratch tile run
    in the startup dead window (7-12us) so the PE clock gate (default
    K=4/8 half rate until ~4us of sustained activity) is fully open
    when chunk 0 lands; chunk 0 loads as two half tiles to land early.
  * Store triggers are deferred past the next chunk's first PSUM copy
    (ScalarE is strict FIFO; a 0.6us DMA trigger ahead of the copy
    that releases the PSUM buffer stalls the PE), and the last chunk
    stores per half to shorten the drain.
  * Input DMAs ride the qSyncDynamicHW ring, output DMAs + weight
    loads the qScalarDynamicHW ring, so loads and stores stream
    concurrently.

Measured on HW: 205us (f32 baseline) -> ~113.5-116.7us run-to-run,
DMA ~93us active (the HBM/DMA-engine floor for 33.6MB at bf16), PE
~92us busy, rel err 2.53e-3 vs the f32 reference (gate 2e-2).
"""

import numpy as np

import concourse.mybir as mybir
import concourse.tile as tile
from concourse import bacc
from concourse.bass_utils import run_bass_kernel_spmd

import os

import ml_dtypes

_BF16 = ml_dtypes.bfloat16

# experiment knobs (test-only; defaults are the shipping config)
_EARLY_STORE_CHUNKS = int(os.environ.get("K_EARLY_STORES", "4"))
_EARLY_DUMMIES = int(os.environ.get("K_EARLY_DUMMIES", "1"))

# Problem constants (hardcoded per harness contract).
B, C, H, W = 8, 32, 512, 512
K = 8  # block size
P = 128  # SBUF partitions
NBR = H // K  # 64 block-rows per channel
CPP = P // NBR  # channels per partition-tile (2)
N_CORES = 8
N_CHUNKS = C // CPP  # 16 tiles per core
HALF = K // 2  # u-slices per PSUM tile (4 banks)

BETA_INT = 0.2 / 8.0  # interior block-rows, interior block-cols
BETA_EDGE = 0.2 / 5.0  # edge block-rows, interior block-cols
GAMMA_INT = 8.0 / 5.0  # count ratio (3*Nr-1)/(2*Nr-1) at Nr=3
GAMMA_EDGE = 5.0 / 3.0  # at Nr=2

_EDGE_PARTS = (0, NBR - 1, NBR, P - 1)  # block-row 0/63 of each channel


def _make_weights():
    """Banded stationary matrices (vertical taps at partition +-1),
    block-diagonal per channel, beta folded in per output partition.
    wc additionally carries the 0.8 center tap. w1/w2 are the w-edge
    column corrections (beta_edge - beta) * (V - I) and * V: applied to
    the first/last 8 columns they retarget the neighbor weights from
    the interior count to the edge count, so PSUM holds the exact
    result everywhere and no post-scaling is needed."""
    beta = np.full(P, BETA_INT, np.float32)
    beta[list(_EDGE_PARTS)] = BETA_EDGE
    beta_e = np.full(P, 0.2 / 5.0, np.float32)  # Nw=2: count=2*Nr-1
    beta_e[list(_EDGE_PARTS)] = 0.2 / 3.0
    dbeta = beta_e - beta

    wf = np.zeros((P, P), np.float32)  # 3-band incl diag: applied to s
    wc = np.zeros((P, P), np.float32)  # 2-band + 0.8 center: unshifted
    w1 = np.zeros((P, P), np.float32)  # dbeta * (V - I): edge cols on x
    w2 = np.zeros((P, P), np.float32)  # dbeta * V: edge cols, x shifted
    for m in range(P):
        for d in (-1, 0, 1):
            k = m + d
            if 0 <= k < P and k // NBR == m // NBR:
                wf[k, m] = beta[m]
                w2[k, m] = dbeta[m]
                if d != 0:
                    wc[k, m] = beta[m]
                    w1[k, m] = dbeta[m]
        wc[m, m] = 0.8
    # pack all four stationaries into one [P, 4P] tensor: a single DMA
    # trigger (~0.7us on the scalar ring) instead of four, so the last
    # weight's completion receipt lands ~2us earlier at startup
    wall = np.concatenate([wf, wc, w1, w2], axis=1)
    return {"wall": wall.astype(_BF16)}


def _build_program():
    f32 = mybir.dt.float32
    bf16 = mybir.dt.bfloat16

    nc = bacc.Bacc("TRN2", target_bir_lowering=False, debug=False,
                   num_devices=N_CORES)

    x_dram = nc.dram_tensor("x", [C, H, W], bf16, kind="ExternalInput")
    y_dram = nc.dram_tensor("y", [C, H, W], bf16, kind="ExternalOutput")
    wall_dram = nc.dram_tensor("wall", [P, 4 * P], bf16, kind="ExternalInput")

    # partition axis = (channel, block-row); free = (u, w)
    x_v = x_dram[:].rearrange("c (bi u) w -> (c bi) u w", u=K)
    y_v = y_dram[:].rearrange("c (bi u) w -> (c bi) u w", u=K)

    with tile.TileContext(nc) as tc:
        with (
            tc.tile_pool(name="wpool", bufs=1) as wpool,
            tc.tile_pool(name="sbuf", bufs=4) as sbuf,
            tc.tile_pool(name="psum", bufs=2, space="PSUM") as psum,
        ):
            # weight loads ride the scalar (store) ring, which is idle
            # at startup, so the sync ring serves chunk loads only
            wall_t = wpool.tile([P, 4 * P], bf16, tag="wall")
            nc.scalar.dma_start(wall_t[:], wall_dram[:])
            wf_t = wall_t[:, 0 * P : 1 * P]
            wc_t = wall_t[:, 1 * P : 2 * P]
            w1_t = wall_t[:, 2 * P : 3 * P]
            w2_t = wall_t[:, 3 * P : 4 * P]

            # HAM warmup: the PE clock gate holds K=4/8 (half rate)
            # until ~4us of sustained activity. Zero the scratch on the
            # GpSimd queue (ready at ~4us, vs ~10us for ScalarE which
            # sits behind DGE config ops) so warmup matmuls run in the
            # ~7-11us window and real matmuls start at full 2.4 GHz.
            scratch = wpool.tile([P, W], bf16, tag="scratch")
            nc.gpsimd.memset(scratch[:], 0.0)
            # 48 LDW+MM pairs at the cold ~107ns pair rate ~= 4.9us of
            # CONTINUOUS activity from ~7.4us: past the ~4.6us HAM
            # unthrottle threshold, and long enough that the PE rolls
            # straight from warmup into chunk-0 work (~11.5us) with no
            # idle gap -- an idle gap decays the HAM credit and with the
            # lighter 2-pass PE schedule the gate then stays shut for
            # 10+us of half-rate matmuls
            uw = psum.tile([P, HALF, W], f32, tag="u")
            for _ in range(48):
                nc.tensor.matmul(uw[:, 0, 0:P], scratch[:, 0:P],
                                 scratch[:, 0:P], start=True, stop=True)

            pending_store = None
            for i in range(N_CHUNKS):
                p0 = i * P
                # steady-state loads stay on the qSyncDynamicHW ring:
                # mixing dependent stores into the same FIFO ring
                # head-of-line-blocks later loads. But the scalar HWDGE
                # ring is idle until the first store (~20us), so the
                # startup burst borrows it: chunk-0 h1 and chunk 1
                # stream there concurrently with chunk-0 h0 / chunk 2
                # on the sync ring, halving time-to-first-compute and
                # building 2 chunks of runahead before stores begin.
                if i == 0 or i == N_CHUNKS - 1:
                    # chunk 0: h1 borrows the idle scalar ring so both
                    # halves land concurrently (only wall+x01 ride the
                    # scalar ring early -- queuing more behind them
                    # delays x01's completion receipt by ~4us).
                    # chunk 15: halves on sync so h0's receipt lands
                    # ~2.5us earlier, shortening the compute tail.
                    xparts = []
                    for hh in range(2):
                        eng = nc.scalar if (i == 0 and hh == 1) else nc.sync
                        t = sbuf.tile([P, HALF, W], bf16, tag=f"x0{hh}")
                        eng.dma_start(
                            t[:],
                            x_v[p0 : p0 + P, hh * HALF : (hh + 1) * HALF],
                        )
                        xparts.append(t[:])
                else:
                    xin = sbuf.tile([P, K, W], bf16, tag="xin")
                    nc.sync.dma_start(xin[:], x_v[p0 : p0 + P])
                    xparts = [xin[:, 0:HALF], xin[:, HALF:K]]

                out_t = sbuf.tile([P, K, W], bf16, tag="out")
                for h in range(2):
                    u0 = h * HALF
                    xp = xparts[h]  # [P, HALF, W] view for this half
                    # separable stencil: the dj=+-1 taps applied to x
                    # equal the dj=0 taps applied to s = x<<8 + x>>8
                    # (zero-filled at the w edges). DVE builds s in bf16
                    # while the PE runs the wc/edge passes, so the PE
                    # does 2 full-width passes per u-slice instead of 3:
                    # per-chunk PE time drops 5.8us -> ~4.9us, below the
                    # 5.4us DMA period, so the PE no longer paces the
                    # pipeline. s depends only on the chunk load, and
                    # nothing else runs on DVE, so the s-builds stream
                    # ahead without coupling to the PSUM drain.
                    # early chunks arrive load-paced ~5.5-6us apart vs
                    # 4.9us of PE work, and the resulting boundary idle
                    # regularly trips a HAM close at ~18us (3.4us of
                    # half-rate matmuls). Two dependency-free dummies at
                    # the chunk top fill the idle: at this point the
                    # PSUM slot is already free (previous copy done) so
                    # they run while the chunk's data is still in
                    # flight; the wc pass's start=True discards them.
                    u = psum.tile([P, HALF, W], f32, tag="u")
                    if _EARLY_DUMMIES and 1 <= i <= 4 and h == 0:
                        for _ in range(2):
                            nc.tensor.matmul(
                                u[:, 0, :], scratch[:, 0:P],
                                scratch[:], start=True, stop=True,
                            )
                    s = sbuf.tile([P, HALF, W], bf16, tag=f"s{h}")
                    nc.vector.tensor_add(
                        s[:, :, K : W - K],
                        xp[:, :, 0 : W - 2 * K],
                        xp[:, :, 2 * K : W],
                    )
                    nc.vector.tensor_copy(s[:, :, 0:K], xp[:, :, K : 2 * K])
                    nc.vector.tensor_copy(
                        s[:, :, W - K : W], xp[:, :, W - 2 * K : W - K]
                    )
                    for uu in range(HALF):
                        # center 0.8 tap + vertical taps, full width
                        nc.tensor.matmul(
                            u[:, uu, :], wc_t, xp[:, uu, :],
                            start=True, stop=False,
                        )
                    # w-edge column corrections: retarget the neighbor
                    # weights to the edge counts so PSUM ends up exact.
                    # Each matmul covers all 4 u-slices via a 2-D moving
                    # access pattern (FD=32); grouped per stationary so
                    # only 2 extra LoadStationary switches per half.
                    # These need only xp, giving DVE slack to finish s.
                    nc.tensor.matmul(
                        u[:, :, 0:K], w1_t, xp[:, :, 0:K],
                        start=False, stop=False,
                    )
                    nc.tensor.matmul(
                        u[:, :, W - K : W], w1_t,
                        xp[:, :, W - K : W],
                        start=False, stop=False,
                    )
                    nc.tensor.matmul(
                        u[:, :, 0:K], w2_t, xp[:, :, K : 2 * K],
                        start=False, stop=False,
                    )
                    nc.tensor.matmul(
                        u[:, :, W - K : W], w2_t,
                        xp[:, :, W - 2 * K : W - K],
                        start=False, stop=False,
                    )
                    for uu in range(HALF):
                        # all six dj=+-1 neighbor taps via s
                        nc.tensor.matmul(
                            u[:, uu, :], wf_t, s[:, uu, :],
                            start=False, stop=(uu == HALF - 1),
                        )
                    if i == N_CHUNKS - 1 and h == 1:
                        # pipeline drain for the very last half:
                        # quarter-sized copy->store pairs on the ScalarE
                        # FIFO so the first store's ~2us completion
                        # receipt overlaps the second quarter's copy
                        for q in range(0, HALF, 2):
                            nc.scalar.copy(
                                out_t[:, u0 + q : u0 + q + 2, :],
                                u[:, q : q + 2, :],
                            )
                            nc.scalar.dma_start(
                                y_v[p0 : p0 + P, u0 + q : u0 + q + 2],
                                out_t[:, u0 + q : u0 + q + 2],
                            )
                        continue
                    # result is complete in PSUM; pure full-width copy +
                    # f32->bf16 cast on ScalarE (closest to PSUM, and
                    # keeps DVE off the critical path)
                    nc.scalar.copy(
                        out_t[:, u0 : u0 + HALF, :],
                        u[:, :, :],
                    )
                    if i < _EARLY_STORE_CHUNKS:
                        # first chunks store per half right after the
                        # copy: the store stream starts ~6us earlier,
                        # shrinking the backlog that drains at
                        # single-ring rate after the loads finish (the
                        # early PE has slack to absorb the 0.65us
                        # trigger on the ScalarE FIFO)
                        nc.scalar.dma_start(
                            y_v[p0 : p0 + P, u0 : u0 + HALF],
                            out_t[:, u0 : u0 + HALF],
                        )
                    elif h == 0 and pending_store is not None:
                        # defer the previous chunk's store trigger to
                        # AFTER this half's copy: the copy (which gates
                        # PSUM buffer release -> PE) jumps the strict
                        # ScalarE FIFO ahead of the 0.6us DMA trigger
                        nc.scalar.dma_start(*pending_store)
                        pending_store = None
                    if i == N_CHUNKS - 1:
                        # drain the pipeline tail per half so the final
                        # store (and its ~2us completion receipt)
                        # starts as early as possible
                        nc.scalar.dma_start(
                            y_v[p0 : p0 + P, u0 : u0 + HALF],
                            out_t[:, u0 : u0 + HALF],
                        )
                # stores ride the scalar HWDGE ring so loads and stores
                # stream concurrently (the sync ring serializes behind
                # slot-paced load triggers; the gpsimd SW-DGE ring tops
                # out at ~130 GB/s -- both measured slower)
                if _EARLY_STORE_CHUNKS <= i < N_CHUNKS - 1:
                    pending_store = (y_v[p0 : p0 + P], out_t[:])
    nc.compile()
    return nc


_CACHE = {}


def _get_program():
    if "nc" not in _CACHE:
        _CACHE["nc"] = _build_program()
        _CACHE["w"] = _make_weights()
    return _CACHE["nc"], _CACHE["w"]


def run(x, trace=False, **spmd_kwargs):
    """x: [B, C, H, W] f32 -> (results object, output [B, C, H, W] f32)."""
    nc, weights = _get_program()
    x = np.ascontiguousarray(x, dtype=np.float32).astype(_BF16)
    in_maps = [{"x": x[i], **weights} for i in range(N_CORES)]
    res = run_bass_kernel_spmd(nc, in_maps, list(range(N_CORES)),
                               trace=trace, **spmd_kwargs)
    out = np.stack([res.results[i]["y"] for i in range(N_CORES)], axis=0)
    return res, out.astype(np.float32)


def kernel(x, kernel_size=8, **_ignored):
    assert int(kernel_size) == K, f"kernel hardcoded for k={K}"
    x = np.asarray(x)
    assert x.shape == (B, C, H, W), x.shape
    _, out = run(x)
    return out


if __name__ == "__main__":
    rng = np.random.default_rng(0)
    x = rng.standard_normal((B, C, H, W), dtype=np.float32)
    out = kernel(x, 8)
    print("out", out.shape, out.dtype, float(np.abs(out).mean()))



# revision 38
# speedup vs baseline: 1.1087x; 1.0653x over previous
"""Trainium2 Bass kernel for block-neighbor "contamination" stencil.

Problem: x [B=8, C=32, H=512, W=512] f32, kernel_size k=8.
The image is a 64x64 grid of 8x8 blocks. For each block, out = 0.8*block +
0.2 * mean(8 neighboring blocks) elementwise over the 8x8 tile, with
zero-padding of the block grid and per-position valid-neighbor counts
(interior 8, edges 5, corners 3).

Equivalent pixel form: a sparse 3x3 stencil with taps at +-8 pixels:
    out[r,w] = 0.8*x[r,w] + beta(r,w) * nsum[r,w]
    nsum[r,w] = sum over (dr,dw) in {-8,0,8}^2, (dr,dw) != (0,0), of
                x[r+dr, w+dw]  (zero pad at image borders)
    beta(r,w) = 0.2 / count(r,w),  count = Nr*Nw - 1,
    Nr/Nw = 2 at the first/last block row/col, else 3.

Strategy (pure data parallel, 1 batch item per NeuronCore, 8 cores).
The problem is HBM-bound, so I/O is done in bf16: the host casts x to
bf16 (16 MiB/core in), the kernel writes bf16 (16 MiB/core out), the
host upcasts to f32. End-to-end rel err ~2.5e-3 (quantization), well
under the 2e-2 gate. The two HWDGE rings sustain ~210-215 GB/s each
when both stream (~425 GB/s aggregate per core), so the ~33.6 MB of
I/O is ~80-94 us of wire time depending on how well the phases pack.

  * Layout: SBUF partition p = (channel-pair, block-row bi); free dim =
    (u = row-within-block 0..7, w 0..511). One partition = one block-row =
    8 consecutive image rows = 8KB contiguous DRAM at bf16.
  * Vertical block-neighbor taps (rows +-8) land at partition +-1 ->
    TensorEngine banded 128x128 matmuls (block-diagonal per channel),
    PSUM f32 accumulation, FD-512 matmuls stream at ~213ns.
  * SEPARABLE horizontal taps: the dj=+-1 taps on x equal the dj=0
    taps on s = x<<8 + x>>8 (zero-filled at w edges), which VectorE
    precomputes in bf16 (~1.5us/half, depends only on the chunk load).
    The PE then does 2 full-width passes per u-slice (wc@x carrying
    the 0.8 center + vertical betas, wf@s carrying all six neighbor
    taps) instead of 3: PE drops to ~4.9us/chunk, below the ~5.4us DMA
    period, so DMA -- not the PE -- paces the steady state and the
    post-load compute tail shrinks by ~2 chunks.
  * w-edge columns (first/last 8) get exact-weight corrections ON the
    PE: two tiny banded stationaries w1 = (beta_edge-beta)*(V-I) on
    x[:, edge] and w2 = (beta_edge-beta)*V on x[:, edge+-8] (FD=32,
    2-D moving AP over all 4 u-slices) accumulate into the edge PSUM
    columns, so PSUM holds the exact result everywhere, the combine is
    a pure full-width ScalarE copy (closest engine to PSUM), and DVE
    carries nothing that depends on the PSUM drain (an earlier variant
    with DVE edge post-scaling serialized DVE->PE and lost 30us).
  * All 4 stationaries are packed into one [P, 4*P] DRAM tensor -> a
    single weight DMA; its completion receipt lands ~2us earlier than
    4 separate DMAs, which gates the first real matmul.
  * Startup: input loads ride qSyncDynamicHW; chunk-0's second half +
    the weights ride the (otherwise idle until ~15us) qScalarDynamicHW
    store ring, so compute can start at ~12.5us. Queuing MORE loads
    behind them delays x01's completion receipt by ~4us (receipts lag
    when a ring streams on) -- measured, hence only those two.
  * HAM warmup: 48 dummy FD-128 matmuls on a GpSimd-memset scratch
    tile fill 7.4->12.3us so the PE clock gate (half rate until ~4.6us
    of sustained activity, re-evaluated per 3.4us epoch) opens right
    as chunk-0 work starts and the credit never decays in between.
    Runs where the gate thrashes open/close land at 112-123us.
  * Stores ride qScalarDynamicHW. Chunks 0-1 store per half right
    after their copy (store stream starts ~15us, shrinking the
    backlog that drains single-ring after loads finish); later chunks
    defer their store trigger past the next chunk's first PSUM copy
    (ScalarE is strict FIFO; a 0.65us trigger ahead of the copy that
    releases PSUM stalls the PE). The last chunk loads/stores in
    halves/quarters to shorten the drain. Routing tail stores onto the
    sync ring was tried: triggers serialize behind slot-paced load
    triggers (lost ~3us) and produced a rare read-before-copy race;
    the gpsimd SW-DGE ring tops out at ~130 GB/s. Both abandoned.

Measured on HW: 205us (f32 single-ring baseline) -> 117.5us (staged
baseline) -> ~100-103us typical (best 99.7us, occasional ~112us
outliers under cross-core HBM arbitration skew), rel err 2.54e-3 vs
the f32 reference (gate 2e-2).
"""

import numpy as np

import concourse.mybir as mybir
import concourse.tile as tile
from concourse import bacc
from concourse.bass_utils import run_bass_kernel_spmd

import ml_dtypes

_BF16 = ml_dtypes.bfloat16

# chunks that store per half immediately (see loop body)
_EARLY_STORE_CHUNKS = 2

# Problem constants (hardcoded per harness contract).
B, C, H, W = 8, 32, 512, 512
K = 8  # block size
P = 128  # SBUF partitions
NBR = H // K  # 64 block-rows per channel
CPP = P // NBR  # channels per partition-tile (2)
N_CORES = 8
N_CHUNKS = C // CPP  # 16 tiles per core
HALF = K // 2  # u-slices per PSUM tile (4 banks)

BETA_INT = 0.2 / 8.0  # interior block-rows, interior block-cols
BETA_EDGE = 0.2 / 5.0  # edge block-rows, interior block-cols
GAMMA_INT = 8.0 / 5.0  # count ratio (3*Nr-1)/(2*Nr-1) at Nr=3
GAMMA_EDGE = 5.0 / 3.0  # at Nr=2

_EDGE_PARTS = (0, NBR - 1, NBR, P - 1)  # block-row 0/63 of each channel


def _make_weights():
    """Banded stationary matrices (vertical taps at partition +-1),
    block-diagonal per channel, beta folded in per output partition.
    wc additionally carries the 0.8 center tap. w1/w2 are the w-edge
    column corrections (beta_edge - beta) * (V - I) and * V: applied to
    the first/last 8 columns they retarget the neighbor weights from
    the interior count to the edge count, so PSUM holds the exact
    result everywhere and no post-scaling is needed."""
    beta = np.full(P, BETA_INT, np.float32)
    beta[list(_EDGE_PARTS)] = BETA_EDGE
    beta_e = np.full(P, 0.2 / 5.0, np.float32)  # Nw=2: count=2*Nr-1
    beta_e[list(_EDGE_PARTS)] = 0.2 / 3.0
    dbeta = beta_e - beta

    wf = np.zeros((P, P), np.float32)  # 3-band incl diag: applied to s
    wc = np.zeros((P, P), np.float32)  # 2-band + 0.8 center: unshifted
    w1 = np.zeros((P, P), np.float32)  # dbeta * (V - I): edge cols on x
    w2 = np.zeros((P, P), np.float32)  # dbeta * V: edge cols, x shifted
    for m in range(P):
        for d in (-1, 0, 1):
            k = m + d
            if 0 <= k < P and k // NBR == m // NBR:
                wf[k, m] = beta[m]
                w2[k, m] = dbeta[m]
                if d != 0:
                    wc[k, m] = beta[m]
                    w1[k, m] = dbeta[m]
        wc[m, m] = 0.8
    # pack all four stationaries into one [P, 4P] tensor: a single DMA
    # trigger (~0.7us on the scalar ring) instead of four, so the last
    # weight's completion receipt lands ~2us earlier at startup
    wall = np.concatenate([wf, wc, w1, w2], axis=1)
    return {"wall": wall.astype(_BF16)}


def _build_program():
    f32 = mybir.dt.float32
    bf16 = mybir.dt.bfloat16

    nc = bacc.Bacc("TRN2", target_bir_lowering=False, debug=False,
                   num_devices=N_CORES)

    x_dram = nc.dram_tensor("x", [C, H, W], bf16, kind="ExternalInput")
    y_dram = nc.dram_tensor("y", [C, H, W], bf16, kind="ExternalOutput")
    wall_dram = nc.dram_tensor("wall", [P, 4 * P], bf16, kind="ExternalInput")

    # partition axis = (channel, block-row); free = (u, w)
    x_v = x_dram[:].rearrange("c (bi u) w -> (c bi) u w", u=K)
    y_v = y_dram[:].rearrange("c (bi u) w -> (c bi) u w", u=K)

    with tile.TileContext(nc) as tc:
        with (
            tc.tile_pool(name="wpool", bufs=1) as wpool,
            tc.tile_pool(name="sbuf", bufs=4) as sbuf,
            tc.tile_pool(name="psum", bufs=2, space="PSUM") as psum,
        ):
            # weight loads ride the scalar (store) ring, which is idle
            # at startup, so the sync ring serves chunk loads only
            wall_t = wpool.tile([P, 4 * P], bf16, tag="wall")
            nc.scalar.dma_start(wall_t[:], wall_dram[:])
            wf_t = wall_t[:, 0 * P : 1 * P]
            wc_t = wall_t[:, 1 * P : 2 * P]
            w1_t = wall_t[:, 2 * P : 3 * P]
            w2_t = wall_t[:, 3 * P : 4 * P]

            # HAM warmup: the PE clock gate holds K=4/8 (half rate)
            # until ~4us of sustained activity. Zero the scratch on the
            # GpSimd queue (ready at ~4us, vs ~10us for ScalarE which
            # sits behind DGE config ops) so warmup matmuls run in the
            # ~7-11us window and real matmuls start at full 2.4 GHz.
            scratch = wpool.tile([P, W], bf16, tag="scratch")
            nc.gpsimd.memset(scratch[:], 0.0)
            # 48 LDW+MM pairs at the cold ~107ns pair rate ~= 4.9us of
            # CONTINUOUS activity from ~7.4us: past the ~4.6us HAM
            # unthrottle threshold, and long enough that the PE rolls
            # straight from warmup into chunk-0 work (~11.5us) with no
            # idle gap -- an idle gap decays the HAM credit and with the
            # lighter 2-pass PE schedule the gate then stays shut for
            # 10+us of half-rate matmuls
            uw = psum.tile([P, HALF, W], f32, tag="u")
            for _ in range(48):
                nc.tensor.matmul(uw[:, 0, 0:P], scratch[:, 0:P],
                                 scratch[:, 0:P], start=True, stop=True)

            pending_store = None
            for i in range(N_CHUNKS):
                p0 = i * P
                # steady-state loads stay on the qSyncDynamicHW ring:
                # mixing dependent stores into the same FIFO ring
                # head-of-line-blocks later loads. But the scalar HWDGE
                # ring is idle until the first store (~20us), so the
                # startup burst borrows it: chunk-0 h1 and chunk 1
                # stream there concurrently with chunk-0 h0 / chunk 2
                # on the sync ring, halving time-to-first-compute and
                # building 2 chunks of runahead before stores begin.
                if i == 0 or i == N_CHUNKS - 1:
                    # chunk 0: h1 borrows the idle scalar ring so both
                    # halves land concurrently (only wall+x01 ride the
                    # scalar ring early -- queuing more behind them
                    # delays x01's completion receipt by ~4us).
                    # chunk 15: halves on sync so h0's receipt lands
                    # ~2.5us earlier, shortening the compute tail.
                    xparts = []
                    for hh in range(2):
                        eng = nc.scalar if (i == 0 and hh == 1) else nc.sync
                        t = sbuf.tile([P, HALF, W], bf16, tag=f"x0{hh}")
                        eng.dma_start(
                            t[:],
                            x_v[p0 : p0 + P, hh * HALF : (hh + 1) * HALF],
                        )
                        xparts.append(t[:])
                else:
                    xin = sbuf.tile([P, K, W], bf16, tag="xin")
                    nc.sync.dma_start(xin[:], x_v[p0 : p0 + P])
                    xparts = [xin[:, 0:HALF], xin[:, HALF:K]]

                out_t = sbuf.tile([P, K, W], bf16, tag="out")
                for h in range(2):
                    u0 = h * HALF
                    xp = xparts[h]  # [P, HALF, W] view for this half
                    # separable stencil: the dj=+-1 taps applied to x
                    # equal the dj=0 taps applied to s = x<<8 + x>>8
                    # (zero-filled at the w edges). DVE builds s in bf16
                    # while the PE runs the wc/edge passes, so the PE
                    # does 2 full-width passes per u-slice instead of 3:
                    # per-chunk PE time drops 5.8us -> ~4.9us, below the
                    # 5.4us DMA period, so the PE no longer paces the
                    # pipeline. s depends only on the chunk load, and
                    # nothing else runs on DVE, so the s-builds stream
                    # ahead without coupling to the PSUM drain.
                    u = psum.tile([P, HALF, W], f32, tag="u")
                    s = sbuf.tile([P, HALF, W], bf16, tag=f"s{h}")
                    nc.vector.tensor_add(
                        s[:, :, K : W - K],
                        xp[:, :, 0 : W - 2 * K],
                        xp[:, :, 2 * K : W],
                    )
                    nc.vector.tensor_copy(s[:, :, 0:K], xp[:, :, K : 2 * K])
                    nc.vector.tensor_copy(
                        s[:, :, W - K : W], xp[:, :, W - 2 * K : W - K]
                    )
                    for uu in range(HALF):
                        # center 0.8 tap + vertical taps, full width
                        nc.tensor.matmul(
                            u[:, uu, :], wc_t, xp[:, uu, :],
                            start=True, stop=False,
                        )
                    # w-edge column corrections: retarget the neighbor
                    # weights to the edge counts so PSUM ends up exact.
                    # Each matmul covers all 4 u-slices via a 2-D moving
                    # access pattern (FD=32); grouped per stationary so
                    # only 2 extra LoadStationary switches per half.
                    # These need only xp, giving DVE slack to finish s.
                    nc.tensor.matmul(
                        u[:, :, 0:K], w1_t, xp[:, :, 0:K],
                        start=False, stop=False,
                    )
                    nc.tensor.matmul(
                        u[:, :, W - K : W], w1_t,
                        xp[:, :, W - K : W],
                        start=False, stop=False,
                    )
                    nc.tensor.matmul(
                        u[:, :, 0:K], w2_t, xp[:, :, K : 2 * K],
                        start=False, stop=False,
                    )
                    nc.tensor.matmul(
                        u[:, :, W - K : W], w2_t,
                        xp[:, :, W - 2 * K : W - K],
                        start=False, stop=False,
                    )
                    for uu in range(HALF):
                        # all six dj=+-1 neighbor taps via s
                        nc.tensor.matmul(
                            u[:, uu, :], wf_t, s[:, uu, :],
                            start=False, stop=(uu == HALF - 1),
                        )
                    if i == N_CHUNKS - 1 and h == 1:
                        # pipeline drain for the very last half:
                        # quarter-sized copy->store pairs on the ScalarE
                        # FIFO so the first store's ~2us completion
                        # receipt overlaps the second quarter's copy
                        for q in range(0, HALF, 2):
                            nc.scalar.copy(
                                out_t[:, u0 + q : u0 + q + 2, :],
                                u[:, q : q + 2, :],
                            )
                            nc.scalar.dma_start(
                                y_v[p0 : p0 + P, u0 + q : u0 + q + 2],
                                out_t[:, u0 + q : u0 + q + 2],
                            )
                        continue
                    # result is complete in PSUM; pure full-width copy +
                    # f32->bf16 cast on ScalarE (closest to PSUM, and
                    # keeps DVE off the critical path)
                    nc.scalar.copy(
                        out_t[:, u0 : u0 + HALF, :],
                        u[:, :, :],
                    )
                    if i < _EARLY_STORE_CHUNKS:
                        # first chunks store per half right after the
                        # copy: the store stream starts ~6us earlier,
                        # shrinking the backlog that drains at
                        # single-ring rate after the loads finish (the
                        # early PE has slack to absorb the 0.65us
                        # trigger on the ScalarE FIFO)
                        nc.scalar.dma_start(
                            y_v[p0 : p0 + P, u0 : u0 + HALF],
                            out_t[:, u0 : u0 + HALF],
                        )
                    elif h == 0 and pending_store is not None:
                        # defer the previous chunk's store trigger to
                        # AFTER this half's copy: the copy (which gates
                        # PSUM buffer release -> PE) jumps the strict
                        # ScalarE FIFO ahead of the 0.6us DMA trigger
                        nc.scalar.dma_start(*pending_store)
                        pending_store = None
                    if i == N_CHUNKS - 1:
                        # drain the pipeline tail per half so the final
                        # store (and its ~2us completion receipt)
                        # starts as early as possible
                        nc.scalar.dma_start(
                            y_v[p0 : p0 + P, u0 : u0 + HALF],
                            out_t[:, u0 : u0 + HALF],
                        )
                # stores ride the scalar HWDGE ring so loads and stores
                # stream concurrently (the sync ring serializes behind
                # slot-paced load triggers; the gpsimd SW-DGE ring tops
                # out at ~130 GB/s -- both measured slower)
                if _EARLY_STORE_CHUNKS <= i < N_CHUNKS - 1:
                    pending_store = (y_v[p0 : p0 + P], out_t[:])
    nc.compile()
    return nc


_CACHE = {}


def _get_program():
    if "nc" not in _CACHE:
        _CACHE["nc"] = _build_program()
        _CACHE["w"] = _make_weights()
    return _CACHE["nc"], _CACHE["w"]


def run(x, trace=False, **spmd_kwargs):
    """x: [B, C, H, W] f32 -> (results object, output [B, C, H, W] f32)."""
    nc, weights = _get_program()
    x = np.ascontiguousarray(x, dtype=np.float32).astype(_BF16)
    in_maps = [{"x": x[i], **weights} for i in range(N_CORES)]
    res = run_bass_kernel_spmd(nc, in_maps, list(range(N_CORES)),
                               trace=trace, **spmd_kwargs)
    out = np.stack([res.results[i]["y"] for i in range(N_CORES)], axis=0)
    return res, out.astype(np.float32)


def kernel(x, kernel_size=8, **_ignored):
    assert int(kernel_size) == K, f"kernel hardcoded for k={K}"
    x = np.asarray(x)
    assert x.shape == (B, C, H, W), x.shape
    _, out = run(x)
    return out


if __name__ == "__main__":
    rng = np.random.default_rng(0)
    x = rng.standard_normal((B, C, H, W), dtype=np.float32)
    out = kernel(x, 8)
    print("out", out.shape, out.dtype, float(np.abs(out).mean()))

